# revision 26
# baseline (speedup 1.0000x reference)
"""Trainium2 Bass kernel for a 3-layer dense-adjacency GCN decoder.

Problem (per batch graph): 3x GCN layer (msg = h@W + b; agg = A @ msg; relu)
followed by output projection + node mask. B=8 graphs of N=2048 nodes,
latent=64, hidden=128, out=64. Batch-parallel: one graph per NeuronCore.

Per-core plan:
  - The host hands each core its graph's adjacency pre-transposed (a pure
    layout permutation; the HW still reads the full fp32 matrix).  A^T is
    DMA'd once, directly into an SBUF-resident buffer, and reused by all
    3 layers (the aggregation contracts over A's column index, which must
    live on partitions).
  - Layer-0 aggregation accumulates its j-steps in DMA arrival order, so
    it finishes with the stream.
  - Features are kept feature-major (h^T: [d, n], fp32). msg^T = W
    (stationary) @ h^T (moving, free-dim 512, float32r single-pass rate);
    bias rides the PSUM->SBUF evacuation as a per-partition ACT bias; PE
    transposes turn msg^T into node-major msg chunks for the aggregation.
  - ReLU rides the aggregation evacuation straight into the next h^T; the
    output projection mirrors the msg path; the node mask is a
    per-partition ACT scale on the final transposed copy.
  - variant "bfloat16": A^T is cast to bf16 during the DMA (SWDGE; the
    HBM read is still the full fp32 matrix), msg chunks are bf16; h and
    the W-matmuls stay f32r.  variant "float32r": A^T stored fp32 and
    read as f32r (TF32-like) by the PE (staged + DVE-rounded to satisfy
    the fp32r producer-rounding rule).  variant "float32": everything
    fp32 (4 cycles/row aggregation; reference-accurate).
"""

import functools
import os

import numpy as np

import concourse.bass as bass
import concourse.bacc as bacc
import concourse.tile as tile
from concourse import mybir
from concourse.bass_utils import run_bass_kernel_spmd

B = 8
N = 2048
NT = N // 128  # 16 partition tiles
LAT = 64
HID = 128
ODIM = 64
N_CORES = 8

F32 = mybir.dt.float32
BF16 = mybir.dt.bfloat16
F32R = mybir.dt.float32r
Act = mybir.ActivationFunctionType


def _c(ap, dt):
    """View an fp32 AP as `dt` for the PE (same 4-byte storage)."""
    return ap if dt == F32 else ap.bitcast(dt)


@functools.lru_cache(maxsize=8)
def _build(variant: str, round_at: bool = False):
    hostcast = variant == "bfloat16h"
    bf16 = variant in ("bfloat16", "bfloat16h")
    # dtype of the A^T / msg storage (the aggregation operands)
    sdt = BF16 if bf16 else F32
    # dtype the W-stationary (msg/proj) matmuls run at
    wdt = F32 if variant == "float32" else F32R
    # dtype the aggregation matmul reads its operands as
    adt = BF16 if bf16 else getattr(mybir.dt, variant)

    nc = bacc.Bacc(None, target_bir_lowering=False, debug=False)

    AT_d = nc.declare_dram_parameter("AT", [N, N],
                                     BF16 if hostcast else F32,
                                     isOutput=False)
    X_d = nc.declare_dram_parameter("X", [N, LAT], F32, isOutput=False)
    MSK_d = nc.declare_dram_parameter("MSK", [NT, 128], F32, isOutput=False)
    W0_d = nc.declare_dram_parameter("W0", [LAT, HID], F32, isOutput=False)
    W1_d = nc.declare_dram_parameter("W1", [HID, HID], F32, isOutput=False)
    W2_d = nc.declare_dram_parameter("W2", [HID, HID], F32, isOutput=False)
    WO_d = nc.declare_dram_parameter("WO", [HID, ODIM], F32, isOutput=False)
    B0_d = nc.declare_dram_parameter("B0", [HID, 1], F32, isOutput=False)
    B1_d = nc.declare_dram_parameter("B1", [HID, 1], F32, isOutput=False)
    B2_d = nc.declare_dram_parameter("B2", [HID, 1], F32, isOutput=False)
    BO_d = nc.declare_dram_parameter("BO", [ODIM, 1], F32, isOutput=False)
    ID_d = nc.declare_dram_parameter("ID", [128, 128], F32, isOutput=False)
    Y_d = nc.declare_dram_parameter("Y", [N, ODIM], F32, isOutput=True)
    Y3 = Y_d[:].rearrange("(t p) f -> p t f", p=128)

    with tile.TileContext(nc) as tc:
        with (
            tc.tile_pool(name="const", bufs=1) as constp,
            tc.tile_pool(name="at", bufs=1) as atp,
            tc.tile_pool(name="stage", bufs=8) as stagep,
            tc.tile_pool(name="ht", bufs=2) as htp,
            tc.tile_pool(name="msg", bufs=2) as msgp,
            tc.tile_pool(name="msgt", bufs=2) as msgtp,
            tc.tile_pool(name="xo", bufs=1) as xop,
            tc.tile_pool(name="tp", bufs=2, space=bass.MemorySpace.PSUM) as tpp,
            tc.tile_pool(name="aggp", bufs=4, space=bass.MemorySpace.PSUM) as aggp,
            tc.tile_pool(name="msum", bufs=2, space=bass.MemorySpace.PSUM) as msump,
        ):
            at_t = atp.tile([128, NT * N], sdt, tag="at")
            at3 = at_t[:].rearrange("p (j c) -> p j c", c=N)

            # ---- constants first (SWDGE), ordered by need ----
            ident = constp.tile([128, 128], F32, tag="ident")
            nc.gpsimd.dma_start(ident[:], ID_d[:])
            xn_t = xop.tile([128, NT * LAT], F32, tag="xo")
            nc.gpsimd.dma_start(
                xn_t[:].rearrange("p (t f) -> p t f", f=LAT),
                X_d[:].rearrange("(t p) f -> p t f", p=128),
            )
            w0_t = constp.tile([LAT, HID], F32, tag="w0")
            nc.gpsimd.dma_start(w0_t[:], W0_d[:])
            b0_t = constp.tile([HID, 1], F32, tag="b0")
            nc.gpsimd.dma_start(b0_t[:], B0_d[:])

            # ---- the A^T stream: straight into the resident buffer ----
            stage = {}
            if hostcast:
                for j in range(NT):
                    nc.sync.dma_start(
                        at3[:, j, :], AT_d[j * 128 : (j + 1) * 128, :]
                    )
            elif bf16:
                for j in range(NT):
                    nc.gpsimd.dma_start(
                        at3[:, j, :], AT_d[j * 128 : (j + 1) * 128, :]
                    )
            elif round_at and variant == "float32r":
                for j in range(NT):
                    for h in range(2):
                        st = stagep.tile([128, N // 2], F32, tag="stage",
                                         name=f"st{j}_{h}")
                        nc.sync.dma_start(
                            st[:], AT_d[j * 128 : (j + 1) * 128,
                                        h * 1024 : (h + 1) * 1024]
                        )
                        stage[(j, h)] = st
            else:
                for j in range(NT):
                    nc.sync.dma_start(
                        at3[:, j, :], AT_d[j * 128 : (j + 1) * 128, :]
                    )

            w1_t = constp.tile([HID, HID], F32, tag="w1")
            nc.gpsimd.dma_start(w1_t[:], W1_d[:])
            b1_t = constp.tile([HID, 1], F32, tag="b1")
            nc.gpsimd.dma_start(b1_t[:], B1_d[:])
            w2_t = constp.tile([HID, HID], F32, tag="w2")
            nc.gpsimd.dma_start(w2_t[:], W2_d[:])
            b2_t = constp.tile([HID, 1], F32, tag="b2")
            nc.gpsimd.dma_start(b2_t[:], B2_d[:])
            wo_t = constp.tile([HID, ODIM], F32, tag="wo")
            nc.gpsimd.dma_start(wo_t[:], WO_d[:])
            bo_t = constp.tile([ODIM, 1], F32, tag="bo")
            nc.gpsimd.dma_start(bo_t[:], BO_d[:])
            mskn_t = constp.tile([NT, 128], F32, tag="mskn")
            nc.gpsimd.dma_start(mskn_t[:], MSK_d[:])

            # identity in the msg storage dtype for the msg transposes
            if bf16:
                ident_s = constp.tile([128, 128], BF16, tag="idents")
                nc.vector.tensor_copy(ident_s[:], ident[:])
            else:
                ident_s = ident

            # round the stationary weights once for the f32r matmuls
            w0_r = constp.tile([LAT, HID], F32, tag="w0r")
            nc.vector.tensor_copy(_c(w0_r[:], wdt), w0_t[:])
            w1_r = constp.tile([HID, HID], F32, tag="w1r")
            nc.vector.tensor_copy(_c(w1_r[:], wdt), w1_t[:])
            w2_r = constp.tile([HID, HID], F32, tag="w2r")
            nc.vector.tensor_copy(_c(w2_r[:], wdt), w2_t[:])
            wo_r = constp.tile([HID, ODIM], F32, tag="wor")
            nc.vector.tensor_copy(_c(wo_r[:], wdt), wo_t[:])

            # mask^T: [128, NT], column t holds mask[t*128 : (t+1)*128]
            mskT = constp.tile([128, NT], F32, tag="mskT")

            def emit_maskT():
                with nc.named_scope("maskT"):
                    mps = tpp.tile([128, 512], F32, tag="tp", name="mask_ps")
                    nc.tensor.transpose(
                        mps[0:128, 0:NT], mskn_t[:], ident[0:NT, 0:NT]
                    )
                    nc.vector.tensor_copy(mskT[:], mps[0:128, 0:NT])

            # X^T -> h0^T  [LAT, N]
            hT0 = htp.tile([128, N], F32, tag="ht")

            def emit_xT():
                with nc.named_scope("xT"):
                    for g in range(4):
                        ps = tpp.tile([128, 512], F32, tag="tp", name=f"xtp{g}")
                        for q in range(4):
                            t = 4 * g + q
                            nc.tensor.transpose(
                                ps[0:LAT, q * 128 : (q + 1) * 128],
                                xn_t[:, t * LAT : (t + 1) * LAT],
                                ident[:],
                            )
                        nc.vector.tensor_copy(
                            _c(hT0[0:LAT, g * 512 : (g + 1) * 512], wdt),
                            ps[0:LAT, 0:512],
                        )

            msgT_tiles = {}

            def emit_msgT(lname, i, hT, din, w_r, b_col):
                """Stage 1: msg^T chunk i = (W stationary) @ h^T[:, i-slice]
                into PSUM, bias on the ACT evacuation to SBUF."""
                with nc.named_scope(lname):
                    mp = msump.tile([128, 512], F32, tag="msum",
                                    name=f"{lname}_mp{i}")
                    nc.tensor.matmul(
                        mp[0:HID, :],
                        _c(w_r[0:din, :], wdt),
                        _c(hT[0:din, i * 512 : (i + 1) * 512], wdt),
                        start=True,
                        stop=True,
                    )
                    msgT = msgtp.tile([128, 512], sdt, tag="msgt",
                                      name=f"{lname}_msgT{i}")
                    if i % 2 == 0:
                        nc.scalar.activation(
                            msgT[:], mp[0:HID, :], Act.Identity, bias=b_col[:]
                        )
                    else:
                        nc.vector.tensor_scalar_add(
                            msgT[:], mp[0:HID, :], b_col[:]
                        )
                    msgT_tiles[(lname, i)] = msgT

            def emit_msg_tp(lname, i, msg_nat):
                """Stage 2: PE-transpose msg^T chunk i into node-major."""
                with nc.named_scope(lname):
                    msgT = msgT_tiles.pop((lname, i))
                    ps = tpp.tile([128, 512], sdt, tag="tp",
                                  name=f"{lname}_tp{i}")
                    for q in range(4):
                        nc.tensor.transpose(
                            ps[:, q * 128 : (q + 1) * 128],
                            msgT[:, q * 128 : (q + 1) * 128],
                            ident_s[:],
                        )
                    nc.vector.tensor_copy(
                        _c(msg_nat[:, i * 512 : (i + 1) * 512], adt), ps[:]
                    )

            def emit_msg_chunk(lname, i, hT, din, w_r, b_col, msg_nat):
                emit_msgT(lname, i, hT, din, w_r, b_col)
                emit_msg_tp(lname, i, msg_nat)

            projT_tiles = {}

            def emit_projT(i, hT):
                with nc.named_scope("proj"):
                    pp = msump.tile([128, 512], F32, tag="msum", name=f"pp{i}")
                    nc.tensor.matmul(
                        pp[0:ODIM, :],
                        _c(wo_r[:], wdt),
                        _c(hT[:, i * 512 : (i + 1) * 512], wdt),
                        start=True,
                        stop=True,
                    )
                    projT = msgtp.tile([128, 512], F32, tag="msgt",
                                       name=f"projT{i}")
                    nc.vector.tensor_scalar_add(
                        projT[0:ODIM, :], pp[0:ODIM, :], bo_t[:]
                    )
                    projT_tiles[i] = projT

            def emit_proj_out(i, out_sb):
                """Transpose proj^T chunk i back node-major with the mask
                as ACT scale, then stream the Y DMA."""
                with nc.named_scope("proj"):
                    projT = projT_tiles.pop(i)
                    ps = tpp.tile([128, 512], F32, tag="tp", name=f"otp{i}")
                    for q in range(4):
                        nc.tensor.transpose(
                            ps[:, q * 64 : (q + 1) * 64],
                            projT[0:ODIM, q * 128 : (q + 1) * 128],
                            ident[0:ODIM, 0:ODIM],
                        )
                    for q in range(4):
                        j = 4 * i + q
                        if q % 2 == 0:
                            nc.scalar.activation(
                                out_sb[:, j * ODIM : (j + 1) * ODIM],
                                ps[:, q * 64 : (q + 1) * 64],
                                Act.Copy,
                                scale=mskT[:, j : j + 1],
                            )
                        else:
                            nc.vector.tensor_scalar_mul(
                                out_sb[:, j * ODIM : (j + 1) * ODIM],
                                ps[:, q * 64 : (q + 1) * 64],
                                mskT[:, j : j + 1],
                            )
                    nc.sync.dma_start(
                        Y3[:, 4 * i : 4 * i + 4, :],
                        out_sb[:].rearrange("p (t f) -> p t f", f=ODIM)[
                            :, 4 * i : 4 * i + 4, :
                        ],
                    )

            def _agg_mm(ap_ps, i, j, msg_nat):
                nc.tensor.matmul(
                    ap_ps[:],
                    _c(msg_nat[:, j * 128 : (j + 1) * 128], adt),
                    _c(at_t[:, j * N + i * 512 : j * N + (i + 1) * 512], adt),
                    start=(j == 0),
                    stop=(j == NT - 1),
                )

            def emit_relu(l, i, ap_ps, hT_next):
                with nc.named_scope(f"relu{l}"):
                    dst = _c(hT_next[:, i * 512 : (i + 1) * 512], wdt)
                    if i % 2 == 0:
                        nc.scalar.activation(dst, ap_ps[:], Act.Relu)
                    else:
                        nc.vector.tensor_scalar_max(dst, ap_ps[:], 0.0)

            # ---- layer 0: aggregation chases the A^T stream ----
            # agg0 chunk i accumulates its j-step as soon as A^T tile j
            # lands; all 4 chunks finish with the stream.
            msg0 = msgp.tile([128, N], sdt, tag="msg", name="msg0")
            out_sb = xop.tile([128, NT * ODIM], F32, tag="xo", name="out_sb")
            hT1 = htp.tile([128, N], F32, tag="ht", name="hT1")
            aps0 = [
                aggp.tile([128, 512], F32, tag="agg", name=f"agg0_{i}")
                for i in range(4)
            ]
            for j in range(NT):
                if j == 0:
                    emit_xT()
                    for g in range(4):
                        emit_msg_chunk("msg0", g, hT0, LAT, w0_r, b0_t, msg0)
                if round_at and variant == "float32r":
                    # f32r rounding pass: DVE/ACT copy the staged halves
                    # into the resident A^T buffer (satisfies the verifier;
                    # agg chunk i only gates on the half covering its cols)
                    for h in range(2):
                        dst = _c(
                            at3[:, j, h * 1024 : (h + 1) * 1024], adt
                        )
                        if h == 0:
                            nc.vector.tensor_copy(dst, stage[(j, h)][:])
                        else:
                            nc.scalar.copy(dst, stage[(j, h)][:])
                with nc.named_scope("agg0"), tc.tile_wait_until(
                    0.010 + 0.0028 * j
                ):
                    for i in range(4):
                        _agg_mm(aps0[i], i, j, msg0)
                # PE-warmth filler: harmless accumulating matmuls keep the
                # HAM clock at 2.4 GHz through the DMA-paced stream phase;
                # the result lands in out_sb and is overwritten by proj.
                if 6 <= j <= 14:
                    if j == 6:
                        fill_ps = msump.tile([128, 512], F32, tag="msum",
                                             name="fill_ps")
                    with nc.named_scope("warm"):
                        for k in range(3):
                            nc.tensor.matmul(
                                fill_ps[:],
                                _c(msg0[:, 0:128], adt),
                                _c(at_t[:, 0:512], adt),
                                start=(j == 6 and k == 0),
                                stop=(j == 14 and k == 2),
                            )
            nc.vector.tensor_copy(out_sb[:, 0:512], fill_ps[:])
            for i in range(4):
                emit_relu(0, i, aps0[i], hT1)

            # ---- layers 1, 2 + projection: per-chunk pipeline with
            # two-stage-delayed follow-on work ----
            msg_cur = msgp.tile([128, N], sdt, tag="msg", name="msg1")
            emit_msgT("msg1", 0, hT1, HID, w1_r, b1_t)
            for g in range(1, 4):
                emit_msgT("msg1", g, hT1, HID, w1_r, b1_t)
                emit_msg_tp("msg1", g - 1, msg_cur)
            emit_msg_tp("msg1", 3, msg_cur)
            hT_cur = hT1
            for li in (1, 2):
                hT_next = htp.tile([128, N], F32, tag="ht", name=f"hT{li + 1}")
                if li < 2:
                    msg_next = msgp.tile([128, N], sdt, tag="msg",
                                         name=f"msg{li + 1}")
                for i in range(4):
                    ap_ps = aggp.tile([128, 512], F32, tag="agg",
                                      name=f"agg{li}_{i}")
                    with nc.named_scope(f"agg{li}"):
                        for j in range(NT):
                            _agg_mm(ap_ps, i, j, msg_cur)
                    emit_relu(li, i, ap_ps, hT_next)
                    if i >= 1:
                        if li < 2:
                            emit_msgT(f"msg{li + 1}", i - 1, hT_next, HID,
                                      w2_r, b2_t)
                        else:
                            if i == 1:
                                emit_maskT()
                            emit_projT(i - 1, hT_next)
                    if i >= 2:
                        if li < 2:
                            emit_msg_tp(f"msg{li + 1}", i - 2, msg_next)
                        else:
                            emit_proj_out(i - 2, out_sb)
                if li < 2:
                    emit_msgT(f"msg{li + 1}", 3, hT_next, HID, w2_r, b2_t)
                    emit_msg_tp(f"msg{li + 1}", 2, msg_next)
                    emit_msg_tp(f"msg{li + 1}", 3, msg_next)
                    msg_cur = msg_next
                else:
                    emit_proj_out(2, out_sb)
                    emit_projT(3, hT_next)
                    emit_proj_out(3, out_sb)
                hT_cur = hT_next

    nc.compile()
    return nc


_EYE = np.eye(128, dtype=np.float32)


def kernel(
    latent_features,
    adjacency_matrix,
    node_mask,
    W0,
    b0,
    W1,
    b1,
    W2,
    b2,
    Wout,
    bout,
    _trace=False,
    _agg_dt=None,
):
    variant = _agg_dt or os.environ.get("GCN_AGG_DT", "float32r")
    round_at = os.environ.get("GCN_ROUND_AT", "1") == "1"
    nc = _build(variant, round_at)

    lat = np.ascontiguousarray(np.asarray(latent_features, dtype=np.float32))
    adj = np.asarray(adjacency_matrix, dtype=np.float32)
    adjT = np.ascontiguousarray(adj.transpose(0, 2, 1))
    if variant == "bfloat16h":
        import ml_dtypes

        adjT = adjT.astype(ml_dtypes.bfloat16)
    msk = np.ascontiguousarray(np.asarray(node_mask, dtype=np.float32))
    w0 = np.ascontiguousarray(np.asarray(W0, dtype=np.float32))
    w1 = np.ascontiguousarray(np.asarray(W1, dtype=np.float32))
    w2 = np.ascontiguousarray(np.asarray(W2, dtype=np.float32))
    wo = np.ascontiguousarray(np.asarray(Wout, dtype=np.float32))
    b0_ = np.asarray(b0, dtype=np.float32).reshape(HID, 1)
    b1_ = np.asarray(b1, dtype=np.float32).reshape(HID, 1)
    b2_ = np.asarray(b2, dtype=np.float32).reshape(HID, 1)
    bo_ = np.asarray(bout, dtype=np.float32).reshape(ODIM, 1)

    in_maps = []
    for c in range(N_CORES):
        in_maps.append(
            {
                "AT": adjT[c],
                "X": lat[c],
                "MSK": msk[c].reshape(NT, 128),
                "W0": w0,
                "W1": w1,
                "W2": w2,
                "WO": wo,
                "B0": b0_,
                "B1": b1_,
                "B2": b2_,
                "BO": bo_,
                "ID": _EYE,
            }
        )

    res = run_bass_kernel_spmd(
        nc, in_maps, core_ids=list(range(N_CORES)), trace=_trace
    )
    out = np.stack([res.results[c]["Y"] for c in range(N_CORES)], axis=0)
    if _trace:
        return out, res
    return out


# revision 27
# speedup vs baseline: 1.0372x; 1.0372x over previous
"""Trainium2 Bass kernel for a 3-layer dense-adjacency GCN decoder.

Problem (per batch graph): 3x GCN layer (msg = h@W + b; agg = A @ msg; relu)
followed by output projection + node mask. B=8 graphs of N=2048 nodes,
latent=64, hidden=128, out=64. Batch-parallel: one graph per NeuronCore.

Per-core plan:
  - The host hands each core its graph's adjacency pre-transposed (a pure
    layout permutation; the HW still reads the full fp32 matrix).  A^T is
    DMA'd once, directly into an SBUF-resident buffer, and reused by all
    3 layers (the aggregation contracts over A's column index, which must
    live on partitions).
  - Layer-0 aggregation accumulates its j-steps in DMA arrival order, so
    it finishes with the stream.
  - Features are kept feature-major (h^T: [d, n], fp32). msg^T = W
    (stationary) @ h^T (moving, free-dim 512, float32r single-pass rate);
    bias rides the PSUM->SBUF evacuation as a per-partition ACT bias; PE
    transposes turn msg^T into node-major msg chunks for the aggregation.
  - ReLU rides the aggregation evacuation straight into the next h^T; the
    output projection mirrors the msg path; the node mask is a
    per-partition ACT scale on the final transposed copy.
  - variant "bfloat16": A^T is cast to bf16 during the DMA (SWDGE; the
    HBM read is still the full fp32 matrix), msg chunks are bf16; h and
    the W-matmuls stay f32r.  variant "float32r": A^T stored fp32 and
    read as f32r (TF32-like) by the PE (staged + DVE-rounded to satisfy
    the fp32r producer-rounding rule).  variant "float32": everything
    fp32 (4 cycles/row aggregation; reference-accurate).
"""

import functools
import os

import numpy as np

import concourse.bass as bass
import concourse.bacc as bacc
import concourse.tile as tile
from concourse import mybir
from concourse.bass_utils import run_bass_kernel_spmd

B = 8
N = 2048
NT = N // 128  # 16 partition tiles
LAT = 64
HID = 128
ODIM = 64
N_CORES = 8

F32 = mybir.dt.float32
BF16 = mybir.dt.bfloat16
F32R = mybir.dt.float32r
Act = mybir.ActivationFunctionType


def _c(ap, dt):
    """View an fp32 AP as `dt` for the PE (same 4-byte storage)."""
    return ap if dt == F32 else ap.bitcast(dt)


@functools.lru_cache(maxsize=8)
def _build(variant: str, round_at: bool = False):
    hostcast = variant == "bfloat16h"
    bf16 = variant in ("bfloat16", "bfloat16h")
    # dtype of the A^T / msg storage (the aggregation operands)
    sdt = BF16 if bf16 else F32
    # dtype the W-stationary (msg/proj) matmuls run at
    wdt = F32 if variant == "float32" else F32R
    # dtype the aggregation matmul reads its operands as
    adt = BF16 if bf16 else getattr(mybir.dt, variant)

    nc = bacc.Bacc(None, target_bir_lowering=False, debug=False)

    AT_d = nc.declare_dram_parameter("AT", [N, N],
                                     BF16 if hostcast else F32,
                                     isOutput=False)
    X_d = nc.declare_dram_parameter("X", [N, LAT], F32, isOutput=False)
    MSK_d = nc.declare_dram_parameter("MSK", [NT, 128], F32, isOutput=False)
    W0_d = nc.declare_dram_parameter("W0", [LAT, HID], F32, isOutput=False)
    W1_d = nc.declare_dram_parameter("W1", [HID, HID], F32, isOutput=False)
    W2_d = nc.declare_dram_parameter("W2", [HID, HID], F32, isOutput=False)
    WO_d = nc.declare_dram_parameter("WO", [HID, ODIM], F32, isOutput=False)
    B0_d = nc.declare_dram_parameter("B0", [HID, 1], F32, isOutput=False)
    B1_d = nc.declare_dram_parameter("B1", [HID, 1], F32, isOutput=False)
    B2_d = nc.declare_dram_parameter("B2", [HID, 1], F32, isOutput=False)
    BO_d = nc.declare_dram_parameter("BO", [ODIM, 1], F32, isOutput=False)
    ID_d = nc.declare_dram_parameter("ID", [128, 128], F32, isOutput=False)
    Y_d = nc.declare_dram_parameter("Y", [N, ODIM], F32, isOutput=True)
    Y3 = Y_d[:].rearrange("(t p) f -> p t f", p=128)

    with tile.TileContext(nc) as tc:
        with (
            tc.tile_pool(name="const", bufs=1) as constp,
            tc.tile_pool(name="at", bufs=1) as atp,
            tc.tile_pool(name="stage", bufs=8) as stagep,
            tc.tile_pool(name="ht", bufs=2) as htp,
            tc.tile_pool(name="msg", bufs=2) as msgp,
            tc.tile_pool(name="msgt", bufs=2) as msgtp,
            tc.tile_pool(name="xo", bufs=1) as xop,
            tc.tile_pool(name="tp", bufs=2, space=bass.MemorySpace.PSUM) as tpp,
            tc.tile_pool(name="aggp", bufs=4, space=bass.MemorySpace.PSUM) as aggp,
            tc.tile_pool(name="msum", bufs=2, space=bass.MemorySpace.PSUM) as msump,
        ):
            at_t = atp.tile([128, NT * N], sdt, tag="at")
            at3 = at_t[:].rearrange("p (j c) -> p j c", c=N)

            # ---- constants first (SWDGE), ordered by need ----
            ident = constp.tile([128, 128], F32, tag="ident")
            nc.gpsimd.dma_start(ident[:], ID_d[:])
            xn_t = xop.tile([128, NT * LAT], F32, tag="xo")
            nc.gpsimd.dma_start(
                xn_t[:].rearrange("p (t f) -> p t f", f=LAT),
                X_d[:].rearrange("(t p) f -> p t f", p=128),
            )
            w0_t = constp.tile([LAT, HID], F32, tag="w0")
            nc.gpsimd.dma_start(w0_t[:], W0_d[:])
            b0_t = constp.tile([HID, 1], F32, tag="b0")
            nc.gpsimd.dma_start(b0_t[:], B0_d[:])

            # ---- the A^T stream: straight into the resident buffer ----
            stage = {}
            if hostcast:
                for j in range(NT):
                    nc.sync.dma_start(
                        at3[:, j, :], AT_d[j * 128 : (j + 1) * 128, :]
                    )
            elif bf16:
                for j in range(NT):
                    nc.gpsimd.dma_start(
                        at3[:, j, :], AT_d[j * 128 : (j + 1) * 128, :]
                    )
            elif round_at and variant == "float32r":
                for j in range(NT):
                    for h in range(2):
                        st = stagep.tile([128, N // 2], F32, tag="stage",
                                         name=f"st{j}_{h}")
                        nc.sync.dma_start(
                            st[:], AT_d[j * 128 : (j + 1) * 128,
                                        h * 1024 : (h + 1) * 1024]
                        )
                        stage[(j, h)] = st
            else:
                for j in range(NT):
                    nc.sync.dma_start(
                        at3[:, j, :], AT_d[j * 128 : (j + 1) * 128, :]
                    )

            w1_t = constp.tile([HID, HID], F32, tag="w1")
            nc.gpsimd.dma_start(w1_t[:], W1_d[:])
            b1_t = constp.tile([HID, 1], F32, tag="b1")
            nc.gpsimd.dma_start(b1_t[:], B1_d[:])
            w2_t = constp.tile([HID, HID], F32, tag="w2")
            nc.gpsimd.dma_start(w2_t[:], W2_d[:])
            b2_t = constp.tile([HID, 1], F32, tag="b2")
            nc.gpsimd.dma_start(b2_t[:], B2_d[:])
            wo_t = constp.tile([HID, ODIM], F32, tag="wo")
            nc.gpsimd.dma_start(wo_t[:], WO_d[:])
            bo_t = constp.tile([ODIM, 1], F32, tag="bo")
            nc.gpsimd.dma_start(bo_t[:], BO_d[:])
            mskn_t = constp.tile([NT, 128], F32, tag="mskn")
            nc.gpsimd.dma_start(mskn_t[:], MSK_d[:])

            # identity in the msg storage dtype for the msg transposes
            if bf16:
                ident_s = constp.tile([128, 128], BF16, tag="idents")
                nc.vector.tensor_copy(ident_s[:], ident[:])
            else:
                ident_s = ident

            # round the stationary weights once for the f32r matmuls
            w0_r = constp.tile([LAT, HID], F32, tag="w0r")
            nc.vector.tensor_copy(_c(w0_r[:], wdt), w0_t[:])
            w1_r = constp.tile([HID, HID], F32, tag="w1r")
            nc.vector.tensor_copy(_c(w1_r[:], wdt), w1_t[:])
            w2_r = constp.tile([HID, HID], F32, tag="w2r")
            nc.vector.tensor_copy(_c(w2_r[:], wdt), w2_t[:])
            wo_r = constp.tile([HID, ODIM], F32, tag="wor")
            nc.vector.tensor_copy(_c(wo_r[:], wdt), wo_t[:])

            # mask^T: [128, NT], column t holds mask[t*128 : (t+1)*128]
            mskT = constp.tile([128, NT], F32, tag="mskT")

            def emit_maskT():
                with nc.named_scope("maskT"):
                    mps = tpp.tile([128, 512], F32, tag="tp", name="mask_ps")
                    nc.tensor.transpose(
                        mps[0:128, 0:NT], mskn_t[:], ident[0:NT, 0:NT]
                    )
                    nc.vector.tensor_copy(mskT[:], mps[0:128, 0:NT])

            # X^T -> h0^T  [LAT, N]
            hT0 = htp.tile([128, N], F32, tag="ht")

            def emit_xT():
                with nc.named_scope("xT"):
                    for g in range(4):
                        ps = tpp.tile([128, 512], F32, tag="tp", name=f"xtp{g}")
                        for q in range(4):
                            t = 4 * g + q
                            nc.tensor.transpose(
                                ps[0:LAT, q * 128 : (q + 1) * 128],
                                xn_t[:, t * LAT : (t + 1) * LAT],
                                ident[:],
                            )
                        nc.vector.tensor_copy(
                            _c(hT0[0:LAT, g * 512 : (g + 1) * 512], wdt),
                            ps[0:LAT, 0:512],
                        )

            msgT_tiles = {}

            def emit_msgT(lname, i, hT, din, w_r, b_col):
                """Stage 1: msg^T chunk i = (W stationary) @ h^T[:, i-slice]
                into PSUM, bias on the ACT evacuation to SBUF."""
                with nc.named_scope(lname):
                    mp = msump.tile([128, 512], F32, tag="msum",
                                    name=f"{lname}_mp{i}")
                    nc.tensor.matmul(
                        mp[0:HID, :],
                        _c(w_r[0:din, :], wdt),
                        _c(hT[0:din, i * 512 : (i + 1) * 512], wdt),
                        start=True,
                        stop=True,
                    )
                    msgT = msgtp.tile([128, 512], sdt, tag="msgt",
                                      name=f"{lname}_msgT{i}")
                    if i % 2 == 0:
                        nc.scalar.activation(
                            msgT[:], mp[0:HID, :], Act.Identity, bias=b_col[:]
                        )
                    else:
                        nc.vector.tensor_scalar_add(
                            msgT[:], mp[0:HID, :], b_col[:]
                        )
                    msgT_tiles[(lname, i)] = msgT

            def emit_msg_tp(lname, i, msg_nat):
                """Stage 2: PE-transpose msg^T chunk i into node-major."""
                with nc.named_scope(lname):
                    msgT = msgT_tiles.pop((lname, i))
                    ps = tpp.tile([128, 512], sdt, tag="tp",
                                  name=f"{lname}_tp{i}")
                    for q in range(4):
                        nc.tensor.transpose(
                            ps[:, q * 128 : (q + 1) * 128],
                            msgT[:, q * 128 : (q + 1) * 128],
                            ident_s[:],
                        )
                    nc.vector.tensor_copy(
                        _c(msg_nat[:, i * 512 : (i + 1) * 512], adt), ps[:]
                    )

            def emit_msg_chunk(lname, i, hT, din, w_r, b_col, msg_nat):
                emit_msgT(lname, i, hT, din, w_r, b_col)
                emit_msg_tp(lname, i, msg_nat)

            projT_tiles = {}

            def emit_projT(i, hT):
                with nc.named_scope("proj"):
                    pp = msump.tile([128, 512], F32, tag="msum", name=f"pp{i}")
                    nc.tensor.matmul(
                        pp[0:ODIM, :],
                        _c(wo_r[:], wdt),
                        _c(hT[:, i * 512 : (i + 1) * 512], wdt),
                        start=True,
                        stop=True,
                    )
                    projT = msgtp.tile([128, 512], F32, tag="msgt",
                                       name=f"projT{i}")
                    nc.vector.tensor_scalar_add(
                        projT[0:ODIM, :], pp[0:ODIM, :], bo_t[:]
                    )
                    projT_tiles[i] = projT

            def emit_proj_out(i, out_sb):
                """Transpose proj^T chunk i back node-major with the mask
                as ACT scale, then stream the Y DMA."""
                with nc.named_scope("proj"):
                    projT = projT_tiles.pop(i)
                    ps = tpp.tile([128, 512], F32, tag="tp", name=f"otp{i}")
                    for q in range(4):
                        nc.tensor.transpose(
                            ps[:, q * 64 : (q + 1) * 64],
                            projT[0:ODIM, q * 128 : (q + 1) * 128],
                            ident[0:ODIM, 0:ODIM],
                        )
                    for q in range(4):
                        j = 4 * i + q
                        if q % 2 == 0:
                            nc.scalar.activation(
                                out_sb[:, j * ODIM : (j + 1) * ODIM],
                                ps[:, q * 64 : (q + 1) * 64],
                                Act.Copy,
                                scale=mskT[:, j : j + 1],
                            )
                        else:
                            nc.vector.tensor_scalar_mul(
                                out_sb[:, j * ODIM : (j + 1) * ODIM],
                                ps[:, q * 64 : (q + 1) * 64],
                                mskT[:, j : j + 1],
                            )
                    nc.sync.dma_start(
                        Y3[:, 4 * i : 4 * i + 4, :],
                        out_sb[:].rearrange("p (t f) -> p t f", f=ODIM)[
                            :, 4 * i : 4 * i + 4, :
                        ],
                    )

            def _agg_mm(ap_ps, i, j, msg_nat):
                nc.tensor.matmul(
                    ap_ps[:],
                    _c(msg_nat[:, j * 128 : (j + 1) * 128], adt),
                    _c(at_t[:, j * N + i * 512 : j * N + (i + 1) * 512], adt),
                    start=(j == 0),
                    stop=(j == NT - 1),
                )

            def emit_relu(l, i, ap_ps, hT_next):
                with nc.named_scope(f"relu{l}"):
                    dst = _c(hT_next[:, i * 512 : (i + 1) * 512], wdt)
                    if i % 2 == 0:
                        nc.scalar.activation(dst, ap_ps[:], Act.Relu)
                    else:
                        nc.vector.tensor_scalar_max(dst, ap_ps[:], 0.0)

            # ---- layer 0: aggregation chases the A^T stream ----
            # agg0 chunk i accumulates its j-step as soon as A^T tile j
            # lands; all 4 chunks finish with the stream.
            msg0 = msgp.tile([128, N], sdt, tag="msg", name="msg0")
            out_sb = xop.tile([128, NT * ODIM], F32, tag="xo", name="out_sb")
            hT1 = htp.tile([128, N], F32, tag="ht", name="hT1")
            aps0 = [
                aggp.tile([128, 512], F32, tag="agg", name=f"agg0_{i}")
                for i in range(4)
            ]
            for j in range(NT):
                if j == 0:
                    emit_xT()
                    for g in range(4):
                        emit_msg_chunk("msg0", g, hT0, LAT, w0_r, b0_t, msg0)
                if round_at and variant == "float32r":
                    # f32r rounding pass: DVE/ACT copy the staged halves
                    # into the resident A^T buffer (satisfies the verifier;
                    # agg chunk i only gates on the half covering its cols)
                    for h in range(2):
                        dst = _c(
                            at3[:, j, h * 1024 : (h + 1) * 1024], adt
                        )
                        if h == 0 or j >= NT - 2:
                            nc.vector.tensor_copy(dst, stage[(j, h)][:])
                        else:
                            nc.scalar.copy(dst, stage[(j, h)][:])
                with nc.named_scope("agg0"), tc.tile_wait_until(
                    0.010 + 0.0028 * j
                ):
                    for i in range(4):
                        _agg_mm(aps0[i], i, j, msg0)
            for i in range(4):
                emit_relu(0, i, aps0[i], hT1)

            # ---- layers 1, 2 + projection: per-chunk pipeline with
            # two-stage-delayed follow-on work ----
            msg_cur = msgp.tile([128, N], sdt, tag="msg", name="msg1")
            emit_msgT("msg1", 0, hT1, HID, w1_r, b1_t)
            for g in range(1, 4):
                emit_msgT("msg1", g, hT1, HID, w1_r, b1_t)
                emit_msg_tp("msg1", g - 1, msg_cur)
            emit_msg_tp("msg1", 3, msg_cur)
            hT_cur = hT1
            for li in (1, 2):
                hT_next = htp.tile([128, N], F32, tag="ht", name=f"hT{li + 1}")
                if li < 2:
                    msg_next = msgp.tile([128, N], sdt, tag="msg",
                                         name=f"msg{li + 1}")
                for i in range(4):
                    ap_ps = aggp.tile([128, 512], F32, tag="agg",
                                      name=f"agg{li}_{i}")
                    with nc.named_scope(f"agg{li}"):
                        for j in range(NT):
                            _agg_mm(ap_ps, i, j, msg_cur)
                    emit_relu(li, i, ap_ps, hT_next)
                    if i >= 1:
                        if li < 2:
                            emit_msgT(f"msg{li + 1}", i - 1, hT_next, HID,
                                      w2_r, b2_t)
                        else:
                            if i == 1:
                                emit_maskT()
                            emit_projT(i - 1, hT_next)
                    if i >= 2:
                        if li < 2:
                            emit_msg_tp(f"msg{li + 1}", i - 2, msg_next)
                        else:
                            emit_proj_out(i - 2, out_sb)
                if li < 2:
                    emit_msgT(f"msg{li + 1}", 3, hT_next, HID, w2_r, b2_t)
                    emit_msg_tp(f"msg{li + 1}", 2, msg_next)
                    emit_msg_tp(f"msg{li + 1}", 3, msg_next)
                    msg_cur = msg_next
                else:
                    emit_proj_out(2, out_sb)
                    emit_projT(3, hT_next)
                    emit_proj_out(3, out_sb)
                hT_cur = hT_next

    nc.compile()
    return nc


_EYE = np.eye(128, dtype=np.float32)


def kernel(
    latent_features,
    adjacency_matrix,
    node_mask,
    W0,
    b0,
    W1,
    b1,
    W2,
    b2,
    Wout,
    bout,
    _trace=False,
    _agg_dt=None,
):
    variant = _agg_dt or os.environ.get("GCN_AGG_DT", "float32r")
    round_at = os.environ.get("GCN_ROUND_AT", "1") == "1"
    nc = _build(variant, round_at)

    lat = np.ascontiguousarray(np.asarray(latent_features, dtype=np.float32))
    adj = np.asarray(adjacency_matrix, dtype=np.float32)
    adjT = np.ascontiguousarray(adj.transpose(0, 2, 1))
    if variant == "bfloat16h":
        import ml_dtypes

        adjT = adjT.astype(ml_dtypes.bfloat16)
    msk = np.ascontiguousarray(np.asarray(node_mask, dtype=np.float32))
    w0 = np.ascontiguousarray(np.asarray(W0, dtype=np.float32))
    w1 = np.ascontiguousarray(np.asarray(W1, dtype=np.float32))
    w2 = np.ascontiguousarray(np.asarray(W2, dtype=np.float32))
    wo = np.ascontiguousarray(np.asarray(Wout, dtype=np.float32))
    b0_ = np.asarray(b0, dtype=np.float32).reshape(HID, 1)
    b1_ = np.asarray(b1, dtype=np.float32).reshape(HID, 1)
    b2_ = np.asarray(b2, dtype=np.float32).reshape(HID, 1)
    bo_ = np.asarray(bout, dtype=np.float32).reshape(ODIM, 1)

    in_maps = []
    for c in range(N_CORES):
        in_maps.append(
            {
                "AT": adjT[c],
                "X": lat[c],
                "MSK": msk[c].reshape(NT, 128),
                "W0": w0,
                "W1": w1,
                "W2": w2,
                "WO": wo,
                "B0": b0_,
                "B1": b1_,
                "B2": b2_,
                "BO": bo_,
                "ID": _EYE,
            }
        )

    res = run_bass_kernel_spmd(
        nc, in_maps, core_ids=list(range(N_CORES)), trace=_trace
    )
    out = np.stack([res.results[c]["Y"] for c in range(N_CORES)], axis=0)
    if _trace:
        return out, res
    return out


# revision 29
# speedup vs baseline: 1.0748x; 1.0363x over previous
"""Trainium2 Bass kernel for a 3-layer dense-adjacency GCN decoder.

Problem (per batch graph): 3x GCN layer (msg = h@W + b; agg = A @ msg; relu)
followed by output projection + node mask. B=8 graphs of N=2048 nodes,
latent=64, hidden=128, out=64. Batch-parallel: one graph per NeuronCore.

Per-core plan:
  - The host hands each core its graph's adjacency pre-transposed (a pure
    layout permutation; the HW still reads the full fp32 matrix).  A^T is
    DMA'd once, directly into an SBUF-resident buffer, and reused by all
    3 layers (the aggregation contracts over A's column index, which must
    live on partitions).
  - Layer-0 aggregation accumulates its j-steps in DMA arrival order, so
    it finishes with the stream.
  - Features are kept feature-major (h^T: [d, n], fp32). msg^T = W
    (stationary) @ h^T (moving, free-dim 512, float32r single-pass rate);
    bias rides the PSUM->SBUF evacuation as a per-partition ACT bias; PE
    transposes turn msg^T into node-major msg chunks for the aggregation.
  - ReLU rides the aggregation evacuation straight into the next h^T; the
    output projection mirrors the msg path; the node mask is a
    per-partition ACT scale on the final transposed copy.
  - variant "bfloat16": A^T is cast to bf16 during the DMA (SWDGE; the
    HBM read is still the full fp32 matrix), msg chunks are bf16; h and
    the W-matmuls stay f32r.  variant "float32r": A^T stored fp32 and
    read as f32r (TF32-like) by the PE (staged + DVE-rounded to satisfy
    the fp32r producer-rounding rule).  variant "float32": everything
    fp32 (4 cycles/row aggregation; reference-accurate).
"""

import functools
import os

import numpy as np

import concourse.bass as bass
import concourse.bacc as bacc
import concourse.tile as tile
from concourse import mybir
from concourse.bass_utils import run_bass_kernel_spmd

B = 8
N = 2048
NT = N // 128  # 16 partition tiles
LAT = 64
HID = 128
ODIM = 64
N_CORES = 8

F32 = mybir.dt.float32
BF16 = mybir.dt.bfloat16
F32R = mybir.dt.float32r
Act = mybir.ActivationFunctionType


def _c(ap, dt):
    """View an fp32 AP as `dt` for the PE (same 4-byte storage)."""
    return ap if dt == F32 else ap.bitcast(dt)


@functools.lru_cache(maxsize=8)
def _build(variant: str, round_at: bool = False):
    hostcast = variant == "bfloat16h"
    bf16 = variant in ("bfloat16", "bfloat16h")
    # dtype of the A^T / msg storage (the aggregation operands)
    sdt = BF16 if bf16 else F32
    # dtype the W-stationary (msg/proj) matmuls run at
    wdt = F32 if variant == "float32" else F32R
    # dtype the aggregation matmul reads its operands as
    adt = BF16 if bf16 else getattr(mybir.dt, variant)

    nc = bacc.Bacc(None, target_bir_lowering=False, debug=False)

    AT_d = nc.declare_dram_parameter("AT", [N, N],
                                     BF16 if hostcast else F32,
                                     isOutput=False)
    X_d = nc.declare_dram_parameter("X", [N, LAT], F32, isOutput=False)
    MSK_d = nc.declare_dram_parameter("MSK", [NT, 128], F32, isOutput=False)
    W0_d = nc.declare_dram_parameter("W0", [LAT, HID], F32, isOutput=False)
    W1_d = nc.declare_dram_parameter("W1", [HID, HID], F32, isOutput=False)
    W2_d = nc.declare_dram_parameter("W2", [HID, HID], F32, isOutput=False)
    WO_d = nc.declare_dram_parameter("WO", [HID, ODIM], F32, isOutput=False)
    B0_d = nc.declare_dram_parameter("B0", [HID, 1], F32, isOutput=False)
    B1_d = nc.declare_dram_parameter("B1", [HID, 1], F32, isOutput=False)
    B2_d = nc.declare_dram_parameter("B2", [HID, 1], F32, isOutput=False)
    BO_d = nc.declare_dram_parameter("BO", [ODIM, 1], F32, isOutput=False)
    ID_d = nc.declare_dram_parameter("ID", [128, 128], F32, isOutput=False)
    Y_d = nc.declare_dram_parameter("Y", [N, ODIM], F32, isOutput=True)
    Y3 = Y_d[:].rearrange("(t p) f -> p t f", p=128)

    with tile.TileContext(nc) as tc:
        with (
            tc.tile_pool(name="const", bufs=1) as constp,
            tc.tile_pool(name="at", bufs=1) as atp,
            tc.tile_pool(name="stage", bufs=8) as stagep,
            tc.tile_pool(name="ht", bufs=2) as htp,
            tc.tile_pool(name="msg", bufs=2) as msgp,
            tc.tile_pool(name="msgt", bufs=2) as msgtp,
            tc.tile_pool(name="xo", bufs=1) as xop,
            tc.tile_pool(name="tp", bufs=2, space=bass.MemorySpace.PSUM) as tpp,
            tc.tile_pool(name="aggp", bufs=4, space=bass.MemorySpace.PSUM) as aggp,
            tc.tile_pool(name="msum", bufs=2, space=bass.MemorySpace.PSUM) as msump,
        ):
            at_t = atp.tile([128, NT * N], sdt, tag="at")
            at3 = at_t[:].rearrange("p (j c) -> p j c", c=N)

            # ---- constants first (SWDGE), ordered by need ----
            ident = constp.tile([128, 128], F32, tag="ident")
            nc.gpsimd.dma_start(ident[:], ID_d[:])
            xn_t = xop.tile([128, NT * LAT], F32, tag="xo")
            nc.gpsimd.dma_start(
                xn_t[:].rearrange("p (t f) -> p t f", f=LAT),
                X_d[:].rearrange("(t p) f -> p t f", p=128),
            )
            w0_t = constp.tile([LAT, HID], F32, tag="w0")
            nc.gpsimd.dma_start(w0_t[:], W0_d[:])
            b0_t = constp.tile([HID, 1], F32, tag="b0")
            nc.gpsimd.dma_start(b0_t[:], B0_d[:])

            # ---- the A^T stream: straight into the resident buffer ----
            stage = {}
            if hostcast:
                for j in range(NT):
                    nc.sync.dma_start(
                        at3[:, j, :], AT_d[j * 128 : (j + 1) * 128, :]
                    )
            elif bf16:
                for j in range(NT):
                    nc.gpsimd.dma_start(
                        at3[:, j, :], AT_d[j * 128 : (j + 1) * 128, :]
                    )
            elif round_at and variant == "float32r":
                # panel-major: agg0 chunk i completes right after panel i,
                # unlocking layer-1 partial work under the stream
                for i in range(4):
                    for j in range(NT):
                        st = stagep.tile([128, N // 4], F32, tag="stage",
                                         name=f"st{j}_{i}")
                        nc.sync.dma_start(
                            st[:], AT_d[j * 128 : (j + 1) * 128,
                                        i * 512 : (i + 1) * 512]
                        )
                        stage[(j, i)] = st
            else:
                for j in range(NT):
                    nc.sync.dma_start(
                        at3[:, j, :], AT_d[j * 128 : (j + 1) * 128, :]
                    )

            w1_t = constp.tile([HID, HID], F32, tag="w1")
            nc.gpsimd.dma_start(w1_t[:], W1_d[:])
            b1_t = constp.tile([HID, 1], F32, tag="b1")
            nc.gpsimd.dma_start(b1_t[:], B1_d[:])
            w2_t = constp.tile([HID, HID], F32, tag="w2")
            nc.gpsimd.dma_start(w2_t[:], W2_d[:])
            b2_t = constp.tile([HID, 1], F32, tag="b2")
            nc.gpsimd.dma_start(b2_t[:], B2_d[:])
            wo_t = constp.tile([HID, ODIM], F32, tag="wo")
            nc.gpsimd.dma_start(wo_t[:], WO_d[:])
            bo_t = constp.tile([ODIM, 1], F32, tag="bo")
            nc.gpsimd.dma_start(bo_t[:], BO_d[:])
            mskn_t = constp.tile([NT, 128], F32, tag="mskn")
            nc.gpsimd.dma_start(mskn_t[:], MSK_d[:])

            # identity in the msg storage dtype for the msg transposes
            if bf16:
                ident_s = constp.tile([128, 128], BF16, tag="idents")
                nc.vector.tensor_copy(ident_s[:], ident[:])
            else:
                ident_s = ident

            # round the stationary weights once for the f32r matmuls
            w0_r = constp.tile([LAT, HID], F32, tag="w0r")
            nc.vector.tensor_copy(_c(w0_r[:], wdt), w0_t[:])
            w1_r = constp.tile([HID, HID], F32, tag="w1r")
            nc.vector.tensor_copy(_c(w1_r[:], wdt), w1_t[:])
            w2_r = constp.tile([HID, HID], F32, tag="w2r")
            nc.vector.tensor_copy(_c(w2_r[:], wdt), w2_t[:])
            wo_r = constp.tile([HID, ODIM], F32, tag="wor")
            nc.vector.tensor_copy(_c(wo_r[:], wdt), wo_t[:])

            # mask^T: [128, NT], column t holds mask[t*128 : (t+1)*128]
            mskT = constp.tile([128, NT], F32, tag="mskT")

            def emit_maskT():
                with nc.named_scope("maskT"):
                    mps = tpp.tile([128, 512], F32, tag="tp", name="mask_ps")
                    nc.tensor.transpose(
                        mps[0:128, 0:NT], mskn_t[:], ident[0:NT, 0:NT]
                    )
                    nc.vector.tensor_copy(mskT[:], mps[0:128, 0:NT])

            # X^T -> h0^T  [LAT, N]
            hT0 = htp.tile([128, N], F32, tag="ht")

            def emit_xT():
                with nc.named_scope("xT"):
                    for g in range(4):
                        ps = tpp.tile([128, 512], F32, tag="tp", name=f"xtp{g}")
                        for q in range(4):
                            t = 4 * g + q
                            nc.tensor.transpose(
                                ps[0:LAT, q * 128 : (q + 1) * 128],
                                xn_t[:, t * LAT : (t + 1) * LAT],
                                ident[:],
                            )
                        nc.vector.tensor_copy(
                            _c(hT0[0:LAT, g * 512 : (g + 1) * 512], wdt),
                            ps[0:LAT, 0:512],
                        )

            msgT_tiles = {}

            def emit_msgT(lname, i, hT, din, w_r, b_col):
                """Stage 1: msg^T chunk i = (W stationary) @ h^T[:, i-slice]
                into PSUM, bias on the ACT evacuation to SBUF."""
                with nc.named_scope(lname):
                    mp = msump.tile([128, 512], F32, tag="msum",
                                    name=f"{lname}_mp{i}")
                    nc.tensor.matmul(
                        mp[0:HID, :],
                        _c(w_r[0:din, :], wdt),
                        _c(hT[0:din, i * 512 : (i + 1) * 512], wdt),
                        start=True,
                        stop=True,
                    )
                    msgT = msgtp.tile([128, 512], sdt, tag="msgt",
                                      name=f"{lname}_msgT{i}")
                    if i % 2 == 0:
                        nc.scalar.activation(
                            msgT[:], mp[0:HID, :], Act.Identity, bias=b_col[:]
                        )
                    else:
                        nc.vector.tensor_scalar_add(
                            msgT[:], mp[0:HID, :], b_col[:]
                        )
                    msgT_tiles[(lname, i)] = msgT

            def emit_msg_tp(lname, i, msg_nat):
                """Stage 2: PE-transpose msg^T chunk i into node-major."""
                with nc.named_scope(lname):
                    msgT = msgT_tiles.pop((lname, i))
                    ps = tpp.tile([128, 512], sdt, tag="tp",
                                  name=f"{lname}_tp{i}")
                    for q in range(4):
                        nc.tensor.transpose(
                            ps[:, q * 128 : (q + 1) * 128],
                            msgT[:, q * 128 : (q + 1) * 128],
                            ident_s[:],
                        )
                    nc.vector.tensor_copy(
                        _c(msg_nat[:, i * 512 : (i + 1) * 512], adt), ps[:]
                    )

            def emit_msg_chunk(lname, i, hT, din, w_r, b_col, msg_nat):
                emit_msgT(lname, i, hT, din, w_r, b_col)
                emit_msg_tp(lname, i, msg_nat)

            projT_tiles = {}

            def emit_projT(i, hT):
                with nc.named_scope("proj"):
                    pp = msump.tile([128, 512], F32, tag="msum", name=f"pp{i}")
                    nc.tensor.matmul(
                        pp[0:ODIM, :],
                        _c(wo_r[:], wdt),
                        _c(hT[:, i * 512 : (i + 1) * 512], wdt),
                        start=True,
                        stop=True,
                    )
                    projT = msgtp.tile([128, 512], F32, tag="msgt",
                                       name=f"projT{i}")
                    nc.vector.tensor_scalar_add(
                        projT[0:ODIM, :], pp[0:ODIM, :], bo_t[:]
                    )
                    projT_tiles[i] = projT

            def emit_proj_out(i, out_sb):
                """Transpose proj^T chunk i back node-major with the mask
                as ACT scale, then stream the Y DMA."""
                with nc.named_scope("proj"):
                    projT = projT_tiles.pop(i)
                    ps = tpp.tile([128, 512], F32, tag="tp", name=f"otp{i}")
                    for q in range(4):
                        nc.tensor.transpose(
                            ps[:, q * 64 : (q + 1) * 64],
                            projT[0:ODIM, q * 128 : (q + 1) * 128],
                            ident[0:ODIM, 0:ODIM],
                        )
                    for q in range(4):
                        j = 4 * i + q
                        if q % 2 == 0:
                            nc.scalar.activation(
                                out_sb[:, j * ODIM : (j + 1) * ODIM],
                                ps[:, q * 64 : (q + 1) * 64],
                                Act.Copy,
                                scale=mskT[:, j : j + 1],
                            )
                        else:
                            nc.vector.tensor_scalar_mul(
                                out_sb[:, j * ODIM : (j + 1) * ODIM],
                                ps[:, q * 64 : (q + 1) * 64],
                                mskT[:, j : j + 1],
                            )
                    nc.sync.dma_start(
                        Y3[:, 4 * i : 4 * i + 4, :],
                        out_sb[:].rearrange("p (t f) -> p t f", f=ODIM)[
                            :, 4 * i : 4 * i + 4, :
                        ],
                    )

            def _agg_mm(ap_ps, i, j, msg_nat):
                nc.tensor.matmul(
                    ap_ps[:],
                    _c(msg_nat[:, j * 128 : (j + 1) * 128], adt),
                    _c(at_t[:, j * N + i * 512 : j * N + (i + 1) * 512], adt),
                    start=(j == 0),
                    stop=(j == NT - 1),
                )

            def emit_relu(l, i, ap_ps, hT_next):
                with nc.named_scope(f"relu{l}"):
                    dst = _c(hT_next[:, i * 512 : (i + 1) * 512], wdt)
                    if i % 2 == 0:
                        nc.scalar.activation(dst, ap_ps[:], Act.Relu)
                    else:
                        nc.vector.tensor_scalar_max(dst, ap_ps[:], 0.0)

            # ---- layer 0 (+ layer-1 partials) ----
            msg0 = msgp.tile([128, N], sdt, tag="msg", name="msg0")
            out_sb = xop.tile([128, NT * ODIM], F32, tag="xo", name="out_sb")
            hT1 = htp.tile([128, N], F32, tag="ht", name="hT1")
            panel = round_at and variant == "float32r"
            aps0 = [
                aggp.tile([128, 512], F32, tag="agg", name=f"agg0_{i}")
                for i in range(4)
            ]
            if panel:
                # Panel-major stream: after panel i lands, agg0 chunk i
                # runs all 16 j-steps, ReLUs, and produces msg1 chunk i;
                # agg1 partial accumulations then run under the stream.
                msg1 = msgp.tile([128, N], sdt, tag="msg", name="msg1")
                hT2 = htp.tile([128, N], F32, tag="ht", name="hT2")
                aps1 = {}
                for i in range(4):
                    if i == 0:
                        emit_xT()
                        for g in range(4):
                            emit_msg_chunk("msg0", g, hT0, LAT,
                                           w0_r, b0_t, msg0)
                    for j in range(NT):
                        dst = _c(at3[:, j, i * 512 : (i + 1) * 512], adt)
                        if j % 2 == 0 or (i == 3 and j >= NT - 2):
                            nc.vector.tensor_copy(dst, stage[(j, i)][:])
                        else:
                            nc.scalar.copy(dst, stage[(j, i)][:])
                    with nc.named_scope("agg0"), tc.tile_wait_until(
                        0.010 + 0.0112 * i
                    ):
                        for j in range(NT):
                            _agg_mm(aps0[i], i, j, msg0)
                    emit_relu(0, i, aps0[i], hT1)
                    emit_msgT("msg1", i, hT1, HID, w1_r, b1_t)
                    emit_msg_tp("msg1", i, msg1)
                    # layer-1 partials now unlocked: panels 0..i resident,
                    # msg1 chunks 0..i ready
                    with nc.named_scope("agg1"):
                        for ip in range(i + 1):
                            if ip not in aps1:
                                aps1[ip] = aggp.tile(
                                    [128, 512], F32, tag="agg",
                                    name=f"agg1_{ip}"
                                )
                            j0 = 0 if ip == i else 4 * i
                            for j in range(j0, 4 * i + 4):
                                _agg_mm(aps1[ip], ip, j, msg1)
                for i in range(4):
                    emit_relu(1, i, aps1[i], hT2)
                hT1 = hT2
            else:
                for j in range(NT):
                    if j == 0:
                        emit_xT()
                        for g in range(4):
                            emit_msg_chunk("msg0", g, hT0, LAT,
                                           w0_r, b0_t, msg0)
                    with nc.named_scope("agg0"), tc.tile_wait_until(
                        0.010 + 0.0028 * j
                    ):
                        for i in range(4):
                            _agg_mm(aps0[i], i, j, msg0)
                for i in range(4):
                    emit_relu(0, i, aps0[i], hT1)

            # ---- layers 1, 2 + projection: per-chunk pipeline with
            # two-stage-delayed follow-on work ----
            if panel:
                msg_cur = msgp.tile([128, N], sdt, tag="msg", name="msg2p")
                emit_msgT("msg2p", 0, hT1, HID, w2_r, b2_t)
                for g in range(1, 4):
                    emit_msgT("msg2p", g, hT1, HID, w2_r, b2_t)
                    emit_msg_tp("msg2p", g - 1, msg_cur)
                emit_msg_tp("msg2p", 3, msg_cur)
                layers = (2,)
            else:
                msg_cur = msgp.tile([128, N], sdt, tag="msg", name="msg1")
                emit_msgT("msg1", 0, hT1, HID, w1_r, b1_t)
                for g in range(1, 4):
                    emit_msgT("msg1", g, hT1, HID, w1_r, b1_t)
                    emit_msg_tp("msg1", g - 1, msg_cur)
                emit_msg_tp("msg1", 3, msg_cur)
                layers = (1, 2)
            hT_cur = hT1
            for li in layers:
                hT_next = htp.tile([128, N], F32, tag="ht", name=f"hT{li + 1}")
                if li < 2:
                    msg_next = msgp.tile([128, N], sdt, tag="msg",
                                         name=f"msg{li + 1}")
                for i in range(4):
                    ap_ps = aggp.tile([128, 512], F32, tag="agg",
                                      name=f"agg{li}_{i}")
                    with nc.named_scope(f"agg{li}"):
                        for j in range(NT):
                            _agg_mm(ap_ps, i, j, msg_cur)
                    emit_relu(li, i, ap_ps, hT_next)
                    if i >= 1:
                        if li < 2:
                            emit_msgT(f"msg{li + 1}", i - 1, hT_next, HID,
                                      w2_r, b2_t)
                        else:
                            if i == 1:
                                emit_maskT()
                            emit_projT(i - 1, hT_next)
                    if i >= 2:
                        if li < 2:
                            emit_msg_tp(f"msg{li + 1}", i - 2, msg_next)
                        else:
                            emit_proj_out(i - 2, out_sb)
                if li < 2:
                    emit_msgT(f"msg{li + 1}", 3, hT_next, HID, w2_r, b2_t)
                    emit_msg_tp(f"msg{li + 1}", 2, msg_next)
                    emit_msg_tp(f"msg{li + 1}", 3, msg_next)
                    msg_cur = msg_next
                else:
                    emit_proj_out(2, out_sb)
                    emit_projT(3, hT_next)
                    emit_proj_out(3, out_sb)
                hT_cur = hT_next

    nc.compile()
    return nc


_EYE = np.eye(128, dtype=np.float32)


def kernel(
    latent_features,
    adjacency_matrix,
    node_mask,
    W0,
    b0,
    W1,
    b1,
    W2,
    b2,
    Wout,
    bout,
    _trace=False,
    _agg_dt=None,
):
    variant = _agg_dt or os.environ.get("GCN_AGG_DT", "float32r")
    round_at = os.environ.get("GCN_ROUND_AT", "1") == "1"
    nc = _build(variant, round_at)

    lat = np.ascontiguousarray(np.asarray(latent_features, dtype=np.float32))
    adj = np.asarray(adjacency_matrix, dtype=np.float32)
    adjT = np.ascontiguousarray(adj.transpose(0, 2, 1))
    if variant == "bfloat16h":
        import ml_dtypes

        adjT = adjT.astype(ml_dtypes.bfloat16)
    msk = np.ascontiguousarray(np.asarray(node_mask, dtype=np.float32))
    w0 = np.ascontiguousarray(np.asarray(W0, dtype=np.float32))
    w1 = np.ascontiguousarray(np.asarray(W1, dtype=np.float32))
    w2 = np.ascontiguousarray(np.asarray(W2, dtype=np.float32))
    wo = np.ascontiguousarray(np.asarray(Wout, dtype=np.float32))
    b0_ = np.asarray(b0, dtype=np.float32).reshape(HID, 1)
    b1_ = np.asarray(b1, dtype=np.float32).reshape(HID, 1)
    b2_ = np.asarray(b2, dtype=np.float32).reshape(HID, 1)
    bo_ = np.asarray(bout, dtype=np.float32).reshape(ODIM, 1)

    in_maps = []
    for c in range(N_CORES):
        in_maps.append(
            {
                "AT": adjT[c],
                "X": lat[c],
                "MSK": msk[c].reshape(NT, 128),
                "W0": w0,
                "W1": w1,
                "W2": w2,
                "WO": wo,
                "B0": b0_,
                "B1": b1_,
                "B2": b2_,
                "BO": bo_,
                "ID": _EYE,
            }
        )

    res = run_bass_kernel_spmd(
        nc, in_maps, core_ids=list(range(N_CORES)), trace=_trace
    )
    out = np.stack([res.results[c]["Y"] for c in range(N_CORES)], axis=0)
    if _trace:
        return out, res
    return out
